# revision 20
# baseline (speedup 1.0000x reference)
"""Trainium2 Bass kernel for nn_AttnLayer (dense_transformer, sum-normalized attention).

Reference computation (per batch b, all fp32):
    d      = in_seq[:,b,:] @ W_in2enc.T + prev_target_seq[:,b,:] @ W_lab2enc.T + (b1+b2)
    S      = d @ E.T                      (E = enc_seq[:,b,:])
    ssum_l = sum_m S[l,m]                 (sum-normalization denominator)
    C      = S @ E
    out    = (C / ssum[:,None]) @ W_enc2in.T + b3

The attention is linear (sum-normalization, no softmax), so S is never
materialized: C = d @ (E^T E) = d @ G with the 512x512 Gram matrix G:
    G    = E-blocks.T @ E                            [e, e']  f32r
    H    = G-blocks.T @ W3T                          [e, o]   f32r  (= G @ W3T)
    d^T  = W1T.T @ X^T + W2T.T @ T^T + bd            [e, l]   f32r
    O    = d^T-blocks.T @ H                          [l, o]   f32r, then *1/ssum + b3

Denominator (exact fp32; ssum suffers catastrophic cancellation, min |ssum|
~0.05 vs ~700 typical, so this path must be fp32 end-to-end from raw inputs):
    ssum = X@v1 + T@v2 + esum.bd,  v1 = W1^T esum,  esum = sum_m E[m,:]
Everything on this path runs as matmuls whose MOVING operand is a [K,1]
column, which costs ~nothing on the PE (cost ~ moving rows only; stationary
loads are free):
  - esum[e]  = ones-column reduction of E chunks over partitions
  - v1,v2    = W^T @ esum-column (accumulated over e-chunks)
  - ssum[l]  = X-chunk.T @ v1-column (accumulated over chunks of both inputs)
HW-verified: fp32 ap-size-1 matmuls are exact-fp32-class; f32r data is
rounded to an 11-bit mantissa AT LOAD (DMA) or at engine write, NOT at PE
read.  So the fp32 inputs are loaded once exactly, the denominator reads
them directly, and cheap engine round-copies produce the f32r versions the
numerator matmuls consume (errors there stay relative to the numerator and
cancel against 1/ssum).

Sharding: data-parallel over batch B=16 across 8 cores (2 batches per core).
"""

import os

os.environ.setdefault("MYCRO_LOCAL_CACHE", "1")

import numpy as np

import concourse.bass as bass
from concourse import bacc
import concourse.mybir as mybir
import concourse.tile as tile
from concourse.bass_utils import run_bass_kernel_spmd

# Problem shape (hardcoded per contract)
L = 1024      # L_in == L_enc
B = 16
D = 512       # D_in == D_enc == D_emb
N_CORES = 8
BPC = B // N_CORES   # batches per core
P = 128
NE = D // P          # 4 chunks of contraction axes of size 512
NM = L // P          # 8 chunks of the L_enc axis
NL = L // P          # 8 chunks of the L_in axis
NLH = 2              # l processed in halves of 512 (moving-operand max for 4-byte)
LH = L // NLH

F32 = mybir.dt.float32
F32R = mybir.dt.float32r

# smallT PSUM column map: [0:32) esum partials (mc,ec), [32:36) v1,
# [36:40) v2, [40] c0, [41:49) ssum
C_ESUM = 0
C_V1 = 32
C_V2 = 36
C_C0 = 40
C_SS = 41


def build_nc():
    nc = bacc.Bacc(None, target_bir_lowering=False, debug=False)

    xT_d = nc.declare_dram_parameter("xT", [BPC, D, L], F32, isOutput=False)
    tT_d = nc.declare_dram_parameter("tT", [BPC, D, L], F32, isOutput=False)
    en_d = nc.declare_dram_parameter("en", [BPC, L, D], F32, isOutput=False)
    w1t_d = nc.declare_dram_parameter("w1t", [D, D], F32R, isOutput=False)  # [i, e]
    w2t_d = nc.declare_dram_parameter("w2t", [D, D], F32R, isOutput=False)  # [j, e]
    w3t_d = nc.declare_dram_parameter("w3t", [D, D], F32R, isOutput=False)  # [e, o]
    w1n_d = nc.declare_dram_parameter("w1n", [D, D], F32, isOutput=False)  # [e, i]
    w2n_d = nc.declare_dram_parameter("w2n", [D, D], F32, isOutput=False)  # [e, j]
    bd_d = nc.declare_dram_parameter("bd", [D], F32, isOutput=False)       # b1 + b2
    b3bc_d = nc.declare_dram_parameter("b3bc", [P, D], F32, isOutput=False)
    ones_d = nc.declare_dram_parameter("ones", [P, 1], F32, isOutput=False)
    ident_d = nc.declare_dram_parameter("ident", [P, P], F32R, isOutput=False)
    out_d = nc.declare_dram_parameter("out", [BPC, L, D], F32, isOutput=True)

    AF = mybir.ActivationFunctionType
    AX = mybir.AxisListType

    with tile.TileContext(nc) as tc:
        with (
            tc.tile_pool(name="wpool", bufs=1) as wpool,
            tc.tile_pool(name="big", bufs=1) as big,
            tc.tile_pool(name="vecs", bufs=1) as vecs,
            tc.tile_pool(name="opool", bufs=4) as opool,
            tc.tile_pool(name="psA", bufs=7, space="PSUM") as psA,
            tc.tile_pool(name="psS", bufs=1, space="PSUM") as psS,
        ):
            # ---- persistent weights / constants ----
            w1t = wpool.tile([P, NE, D], F32R, name="w1t")   # [i%128, i//128, e]
            w2t = wpool.tile([P, NE, D], F32R, name="w2t")
            w3t = wpool.tile([P, NE, D], F32R, name="w3t")   # [e%128, e//128, o]
            w1n = wpool.tile([P, NE, D], F32, name="w1n")    # [e%128, e//128, i]
            w2n = wpool.tile([P, NE, D], F32, name="w2n")
            bd_sb = wpool.tile([P, NE], F32, name="bd_sb")
            b3bc = wpool.tile([P, D], F32, name="b3bc")
            ones = wpool.tile([P, 1], F32, name="ones")
            ident = wpool.tile([P, P], F32R, name="ident")

            # Small constants on the ACT queue (tiny transfers).
            nc.scalar.dma_start(out=ones, in_=ones_d[:, :])
            nc.scalar.dma_start(out=ident, in_=ident_d[:, :])
            nc.scalar.dma_start(
                out=bd_sb, in_=bd_d.rearrange("(c p) -> p c", p=P))
            nc.scalar.dma_start(out=b3bc, in_=b3bc_d[:, :])

            # ---- single ordered bulk-load queue (SP), in need order.
            # Slot plan (16 KiB each, reused across lifetimes):
            #   slot_en : en0 F32   -> tT1 F32 (en0 dies after esum0+rounds)
            #   slot_en1: en1 F32 (own slot)
            #   slot_enr: en0_r -> en1_r -> xT1 F32 (Gram is the last reader)
            #   slot_x/t: xT0/tT0 F32 (die after ss0)
            #   slot_xr/tr: xT0_r/tT0_r -> xT1_r/tT1_r
            en0 = big.tile([P, NM, D], F32, name="en0", tag="slot_en")
            en1 = big.tile([P, NM, D], F32, name="en1", tag="slot_en1")
            xT0 = big.tile([P, NE, L], F32, name="xT0", tag="slot_x")
            tT0 = big.tile([P, NE, L], F32, name="tT0", tag="slot_t")
            for mc in range(NM):
                eng = nc.gpsimd if mc == 0 else nc.sync
                eng.dma_start(
                    out=en0[:, mc, :], in_=en_d[0, mc * P : (mc + 1) * P, :])
            for mc in range(NM):
                nc.sync.dma_start(
                    out=en1[:, mc, :], in_=en_d[1, mc * P : (mc + 1) * P, :])
            for k in range(NE):
                nc.sync.dma_start(
                    out=w1t[:, k, :], in_=w1t_d[k * P : (k + 1) * P, :])
                nc.sync.dma_start(
                    out=w2t[:, k, :], in_=w2t_d[k * P : (k + 1) * P, :])
                nc.sync.dma_start(out=xT0[:, k, :], in_=xT_d[0, k * P : (k + 1) * P, :])
                nc.sync.dma_start(out=tT0[:, k, :], in_=tT_d[0, k * P : (k + 1) * P, :])
            nc.sync.dma_start(
                out=w3t, in_=w3t_d.rearrange("(c p) e -> p c e", p=P))
            nc.sync.dma_start(
                out=w1n, in_=w1n_d.rearrange("(c p) e -> p c e", p=P))
            nc.sync.dma_start(
                out=w2n, in_=w2n_d.rearrange("(c p) e -> p c e", p=P))
            en_r = [
                big.tile([P, NM, D], F32R, name="en_r0", tag="slot_enr"),
                big.tile([P, NM, D], F32R, name="en_r1", tag="slot_enr"),
            ]
            en = [en0, en1]
            xT = [xT0, None]
            tT = [tT0, None]

            # one PSUM bank of packed column accumulators for BOTH batches:
            # per batch base b*49: +0:32 esum partials, +32:36 v1, +36:40 v2,
            # +40 c0, +41:49 ssum
            smallT = psS.tile([P, 2 * 49], F32, name="smallT")

            def sT(b, c):
                return smallT[:, 49 * b + c : 49 * b + c + 1]

            # ---- phase helpers (engine roles: Pool = round-copies only,
            # DVE = adds/rcols/consolidation, ACT = psum copies + scales) ----
            G_sb = [None, None]
            dT_t = [None, None]
            H_sb = [None, None]
            xr_t = [None, None]
            tr_t = [None, None]
            esum_sb = [None, None]
            v1c = [None, None]
            v2c = [None, None]
            c0c = [None, None]
            rcols = [None, None]

            def phase_gram(b):
                # per-chunk: 4 esum-partial ap1s (fp32), Pool round-copy,
                # then the chunk's 4 Gram matmuls (mc-outer, 4-bank ILP)
                enb = en[b]
                G_sb[b] = big.tile([P, NE, D], F32R, name=f"G_sb{b}", tag=f"slot_G{b}")
                g_ps = [psA.tile([P, D], F32, name=f"g_ps{b}{gc}", tag="acc")
                        for gc in range(NE)]
                for mc in range(NM):
                    for ec in range(NE):
                        nc.tensor.matmul(
                            sT(b, C_ESUM + 4 * mc + ec),
                            enb[:, mc, ec * P : (ec + 1) * P],
                            ones,
                            start=True, stop=True,
                            skip_group_check=True,
                        )
                    nc.gpsimd.tensor_copy(en_r[b][:, mc, :], enb[:, mc, :])
                    # G is symmetric: compute upper-triangle row segments only
                    # (rows 0/3 full to keep every segment's free size >= 256,
                    # the f32r 1-cycle/row threshold)
                    for gc, lo in ((0, 0), (1, P), (2, 2 * P), (3, 0)):
                        nc.tensor.matmul(
                            g_ps[gc][:, 0 : D - lo],
                            en_r[b][:, mc, gc * P : (gc + 1) * P],
                            en_r[b][:, mc, lo:],
                            start=(mc == 0), stop=(mc == NM - 1),
                        )
                for gc, lo in ((0, 0), (1, P), (2, 2 * P), (3, 0)):
                    nc.scalar.activation(
                        G_sb[b][:, gc, lo:], g_ps[gc][:, 0 : D - lo],
                        AF.Copy, bias=0.0)
            def phase_gram_fix(b):
                # missing lower blocks via PE transpose of the mirrored ones:
                # (1,0)<-(0,1), (2,0)<-(0,2), (2,1)<-(1,2).  Emitted right
                # before the H phase so the ACT row copies are long done.
                for dst_r, dst_c, src_r, src_c in (
                        (1, 0, 0, 1), (2, 0, 0, 2), (2, 1, 1, 2)):
                    tp = psA.tile([P, P], F32R, name=f"tp{b}{dst_r}{dst_c}",
                                  tag="acc")
                    nc.tensor.transpose(
                        tp, G_sb[b][:, src_r, src_c * P : (src_c + 1) * P],
                        ident)
                    nc.scalar.activation(
                        G_sb[b][:, dst_r, dst_c * P : (dst_c + 1) * P], tp,
                        AF.Copy, bias=0.0)

            def phase_dT(b):
                # Pool round-copies in DMA-arrival order, then d^T k-outer in
                # arrival order (x0,t0,x1,t1,...) across 4 interleaved banks
                xr_t[b] = big.tile([P, NE, L], F32R, name=f"xT_r{b}", tag="slot_xr")
                tr_t[b] = big.tile([P, NE, L], F32R, name=f"tT_r{b}", tag="slot_tr")
                for k in range(NE):
                    nc.gpsimd.tensor_copy(xr_t[b][:, k, :], xT[b][:, k, :])
                    nc.vector.tensor_copy(tr_t[b][:, k, :], tT[b][:, k, :])
                dT_t[b] = big.tile([P, NE, L], F32R, name=f"dT{b}", tag="slot_d")
                K_ARRIVAL = [0, 4, 1, 5, 2, 6, 3, 7]
                for lh in range(NLH):
                    d_ps = [psA.tile([P, LH], F32, name=f"d_ps{b}{ec}", tag="acc")
                            for ec in range(NE)]
                    for i, k in enumerate(K_ARRIVAL):
                        w = w1t if k < NE else w2t
                        src = xr_t[b] if k < NE else tr_t[b]
                        for ec in range(NE):
                            nc.tensor.matmul(
                                d_ps[ec],
                                w[:, k % NE, ec * P : (ec + 1) * P],
                                src[:, k % NE, lh * LH : (lh + 1) * LH],
                                start=(i == 0), stop=(i == 2 * NE - 1),
                            )
                    for ec in range(NE):
                        nc.vector.tensor_scalar_add(
                            dT_t[b][:, ec, lh * LH : (lh + 1) * LH], d_ps[ec],
                            bd_sb[:, ec : ec + 1],
                        )

            def phase_H(b):
                H_sb[b] = big.tile([P, NE, D], F32R, name=f"H_sb{b}", tag="slot_H")
                for hc in range(NE):
                    h_ps = psA.tile([P, D], F32, name=f"h_ps{b}", tag="acc")
                    for kc in range(NE):
                        nc.tensor.matmul(
                            h_ps,
                            G_sb[b][:, kc, hc * P : (hc + 1) * P],
                            w3t[:, kc, :],
                            start=(kc == 0), stop=(kc == NE - 1),
                        )
                    nc.scalar.activation(
                        H_sb[b][:, hc, :], h_ps, AF.Copy, bias=0.0)

            c0B = [None, None]

            def phase_denom_prep(b):
                # DVE esum consolidation + c0 prep (inputs ready right after
                # the batch's Gram/esum phase)
                esum_sb[b] = vecs.tile([P, NE], F32, name=f"esum_sb{b}")
                nc.vector.tensor_copy(
                    esum_sb[b], smallT[:, 49 * b + C_ESUM : 49 * b + C_ESUM + 4])
                for mc in range(1, NM):
                    nc.vector.tensor_add(
                        esum_sb[b], esum_sb[b],
                        smallT[:, 49 * b + C_ESUM + 4 * mc
                               : 49 * b + C_ESUM + 4 * mc + 4])
                c0t = vecs.tile([P, 1], F32, name=f"c0t{b}")
                c0m = vecs.tile([P, NE], F32, name=f"c0m{b}")
                c0B[b] = vecs.tile([P, P], F32, name=f"c0B{b}")
                nc.vector.tensor_mul(c0m, bd_sb, esum_sb[b])
                nc.vector.reduce_sum(c0t, c0m, axis=AX.X)
                nc.vector.tensor_copy(c0B[b], c0t.broadcast_to([P, P]))

            def phase_denom(b):
                # PE v/c0 ap1s, ACT col copies, PE ssum ap1s, DVE rcols
                for ic in range(NE):
                    for ec in range(NE):
                        nc.tensor.matmul(
                            sT(b, C_V1 + ic),
                            w1n[:, ec, ic * P : (ic + 1) * P],
                            esum_sb[b][:, ec : ec + 1],
                            start=(ec == 0), stop=(ec == NE - 1),
                            skip_group_check=True,
                        )
                for ic in range(NE):
                    for ec in range(NE):
                        nc.tensor.matmul(
                            sT(b, C_V2 + ic),
                            w2n[:, ec, ic * P : (ic + 1) * P],
                            esum_sb[b][:, ec : ec + 1],
                            start=(ec == 0), stop=(ec == NE - 1),
                            skip_group_check=True,
                        )
                nc.tensor.matmul(
                    sT(b, C_C0), c0B[b], ones, start=True, stop=True,
                    skip_group_check=True,
                )
                v1c[b] = vecs.tile([P, NE], F32, name=f"v1c{b}")
                v2c[b] = vecs.tile([P, NE], F32, name=f"v2c{b}")
                c0c[b] = vecs.tile([P, 1], F32, name=f"c0c{b}")
                nc.scalar.activation(
                    v1c[b], smallT[:, 49 * b + C_V1 : 49 * b + C_V1 + NE],
                    AF.Copy, bias=0.0)
                nc.scalar.activation(
                    v2c[b], smallT[:, 49 * b + C_V2 : 49 * b + C_V2 + NE],
                    AF.Copy, bias=0.0)
                nc.scalar.activation(
                    c0c[b], smallT[:, 49 * b + C_C0 : 49 * b + C_C0 + 1],
                    AF.Copy, bias=0.0)
                sc_sb = vecs.tile([P, NL], F32, name=f"sc_sb{b}")
                rcols[b] = vecs.tile([P, NL], F32, name=f"rcols{b}")
                for lc in range(NL):
                    for k in range(2 * NE):
                        data = xT[b] if k < NE else tT[b]
                        vcol = v1c[b] if k < NE else v2c[b]
                        nc.tensor.matmul(
                            sT(b, C_SS + lc),
                            data[:, k % NE, lc * P : (lc + 1) * P],
                            vcol[:, k % NE : k % NE + 1],
                            start=(k == 0), stop=(k == 2 * NE - 1),
                            skip_group_check=True,
                        )
                for lc in range(NL):
                    nc.vector.tensor_scalar_add(
                        sc_sb[:, lc : lc + 1], sT(b, C_SS + lc), c0c[b])
                    nc.vector.reciprocal(
                        rcols[b][:, lc : lc + 1], sc_sb[:, lc : lc + 1])

            def phase_O(b):
                for lc in range(NL):
                    o_ps = psA.tile([P, D], F32, name=f"o_ps{b}", tag="acc")
                    for ec in range(NE):
                        nc.tensor.matmul(
                            o_ps,
                            dT_t[b][:, ec, lc * P : (lc + 1) * P],
                            H_sb[b][:, ec, :],
                            start=(ec == 0), stop=(ec == NE - 1),
                        )
                    o_sb = opool.tile([P, D], F32, name="o_sb")
                    nc.scalar.activation(
                        o_sb, o_ps, AF.Copy, bias=0.0,
                        scale=rcols[b][:, lc : lc + 1])
                    nc.vector.tensor_add(o_sb, o_sb, b3bc)
                    nc.sync.dma_start(
                        out=out_d[b, lc * P : (lc + 1) * P, :], in_=o_sb)

            # ---- global PE schedule: both Grams first (batch-1 Gram fills
            # batch-0's load window), then per-batch pipelines ----
            phase_gram(0)
            phase_denom_prep(0)
            phase_gram(1)
            phase_denom_prep(1)
            # batch-1 inputs reuse slot_enr/slot_en; the dmas are emitted
            # here so the tag-ring (emission) order matches the lifetimes:
            # en_r0 -> en_r1 -> xT1, and en0 -> tT1.
            xT[1] = big.tile([P, NE, L], F32, name="xT1", tag="slot_enr")
            tT[1] = big.tile([P, NE, L], F32, name="tT1", tag="slot_en")
            for k in range(NE):
                nc.sync.dma_start(out=xT[1][:, k, :], in_=xT_d[1, k * P : (k + 1) * P, :])
                nc.sync.dma_start(out=tT[1][:, k, :], in_=tT_d[1, k * P : (k + 1) * P, :])
            phase_dT(0)
            phase_gram_fix(0)
            phase_H(0)
            phase_denom(0)
            phase_O(0)
            phase_dT(1)
            phase_gram_fix(1)
            phase_H(1)
            phase_denom(1)
            phase_O(1)

    nc.finalize()
    return nc


def _make_in_maps(in_seq, enc_seq, prev_target_seq, W_in2enc, b_in2enc,
                  W_lab2enc, b_lab2enc, W_enc2in, b_enc2in):
    f32 = np.float32
    w1t = np.ascontiguousarray(np.asarray(W_in2enc, f32).T)   # [i, e]
    w2t = np.ascontiguousarray(np.asarray(W_lab2enc, f32).T)  # [j, e]
    w3t = np.ascontiguousarray(np.asarray(W_enc2in, f32).T)   # [e, o]
    w1n = np.ascontiguousarray(np.asarray(W_in2enc, f32))
    w2n = np.ascontiguousarray(np.asarray(W_lab2enc, f32))
    bd = np.ascontiguousarray(np.asarray(b_in2enc, f32) + np.asarray(b_lab2enc, f32))
    b3bc = np.ascontiguousarray(np.broadcast_to(np.asarray(b_enc2in, f32), (P, D)))
    ones = np.ones((P, 1), f32)
    ident = np.eye(P, dtype=f32)

    in_maps = []
    for c in range(N_CORES):
        bs = slice(c * BPC, (c + 1) * BPC)
        x = np.asarray(in_seq[:, bs, :], f32)
        t = np.asarray(prev_target_seq[:, bs, :], f32)
        e = np.asarray(enc_seq[:, bs, :], f32)
        in_maps.append({
            "xT": np.ascontiguousarray(x.transpose(1, 2, 0)),
            "tT": np.ascontiguousarray(t.transpose(1, 2, 0)),
            "en": np.ascontiguousarray(e.transpose(1, 0, 2)),
            "w1t": w1t, "w2t": w2t, "w3t": w3t, "w1n": w1n, "w2n": w2n,
            "bd": bd, "b3bc": b3bc, "ones": ones, "ident": ident,
        })
    return in_maps


_NC_CACHE = {}


def _get_nc():
    if "nc" not in _NC_CACHE:
        _NC_CACHE["nc"] = build_nc()
    return _NC_CACHE["nc"]


def kernel(**inputs):
    in_maps = _make_in_maps(**inputs)
    nc = _get_nc()
    res = run_bass_kernel_spmd(nc, in_maps, core_ids=list(range(N_CORES)))
    out = np.empty((L, B, D), np.float32)
    for c in range(N_CORES):
        per_core = res.results[c]["out"]  # (BPC, L, D)
        for j in range(BPC):
            out[:, c * BPC + j, :] = per_core[j]
    return out


def kernel_sim(core_id=0, **inputs):
    """CoreSim validation path: simulate one core, return its (BPC, L, D) output."""
    from concourse.bass_interp import CoreSim

    in_maps = _make_in_maps(**inputs)
    nc = _get_nc()
    sim = CoreSim(nc)
    for name, val in in_maps[core_id].items():
        sim.tensor(name)[:] = val
    sim.simulate(check_with_hw=False)
    return np.array(sim.tensor("out"))
